# revision 1
# baseline (speedup 1.0000x reference)
"""BitSwiGLU Trainium2 kernel — tensor-parallel over hidden, 8 NeuronCores.

Math (per bit_linear, forward values):
    gamma_x = clip(max|x_row|, 1e-5);  k = rne(x * 127/gamma_x)  in [-127,127]
    gamma_w = clip(mean|w|, 1e-5);    t = sign(w) * (|w| > 0.5*gamma_w)
    y = (k @ t.T) * (gamma_x*gamma_w/127) + b

k and t are small integers, exactly representable in bf16; the TensorEngine
accumulates bf16 products in fp32 PSUM, so k @ t.T is EXACT integer math at
bf16 speed. Ternarization runs as t2 = sign(w-thr)+sign(w+thr) in {-2,0,2}
(split between ScalarE and VectorE); the factor 2 is folded into the
eviction scales.

Sharding (vs. the data-parallel baseline, which re-read all 200MB of f32
weights on every core): hidden is sharded 8 ways.
  - Each core holds 1/8 of gate/val/out weights (host passes them
    pre-transposed to the matmul-friendly [contract, free] layout), so
    per-core weight traffic drops 8x.
  - Every core reads the FULL x and quantizes all tokens redundantly,
    one 256-token quarter-block at a time, feeding mm1's lhsT straight
    from SBUF. The redundant VectorE work is cheap and fully pipelined
    under mm1; it avoids any cross-core coupling on the critical prefix
    (an earlier kxT-AllGather design kept amplifying inter-core skew).
  - Global gamma_w for each weight = tiny AllReduce of per-core |w| sums.
  - mm1: each core computes h[:, its 1024 hidden] for ALL 8192 tokens.
    Per 1024-token block r: per-token max|h| over the local hidden slice is
    AllReduce(max)-ed (4KB), h is re-quantized to bf16 integer levels with
    the exact global gamma_h, and an AllToAll gives core j the fully
    assembled quantized h rows for its 128-token sub-chunk of block r.
  - mm2 is then fully local (token-parallel) against the AllGathered bf16
    ternary out_w — no reduction collective on the tail, output is exact
    fp32. Everything that depends on the LAST AllToAll (token block r=7)
    is emitted after mm2's first output-column pass: DMA queues are FIFO,
    so an early-emitted load waiting on that collective would head-of-line
    block the mm2 weight streams emitted behind it. mm2's lhsT (khT) is
    assembled with transpose-DMA straight from the AllToAll DRAM output.
  Tile-pool scopes are arranged so that concurrent phases never reuse each
  other's SBUF addresses (address reuse inserts WAR syncs that serialize
  phases at runtime).

Token ownership: core i owns global tokens {r*1024 + i*128 + [0,128) for
r in 0..7}; the host wrapper re-interleaves the 8 per-core outputs.
"""

import numpy as np

import concourse.mybir as mybir
import concourse.tile as tile
from concourse import bacc
from concourse import bass_isa
from concourse.bass_utils import run_bass_kernel_spmd

F32 = mybir.dt.float32
BF16 = mybir.dt.bfloat16
AF = mybir.ActivationFunctionType
OP = mybir.AluOpType
AX = mybir.AxisListType

MAGIC = 12582912.0  # 1.5 * 2**23 : (v + MAGIC) - MAGIC == rne(v) for |v| < 2**22

N_CORES = 8
D = 2048            # d_model
H = 8192            # hidden (full)
HL = H // N_CORES   # 1024 hidden per core
T = 8192            # total tokens
TL = T // N_CORES   # 1024 tokens owned per core
KD = D // 128       # 16 contraction chunks, mm1
KHL = HL // 128     # 8  chunks of the local hidden slice
KH = H // 128       # 64 contraction chunks, mm2
RLAST = N_CORES - 1
RG = [list(range(N_CORES))]


def _build():
    nc = bacc.Bacc("TRN2", target_bir_lowering=False, debug=False,
                   num_devices=N_CORES)
    x_d = nc.dram_tensor("x", [T, D], F32, kind="ExternalInput")
    gwT_d = nc.dram_tensor("gwT", [D, HL], F32, kind="ExternalInput")
    vwT_d = nc.dram_tensor("vwT", [D, HL], F32, kind="ExternalInput")
    owT_d = nc.dram_tensor("owT", [HL, D], F32, kind="ExternalInput")
    sel_d = nc.dram_tensor("sel8", [1, N_CORES], F32, kind="ExternalInput")
    out_d = nc.dram_tensor("out", [TL, D], F32, kind="ExternalOutput")

    with tile.TileContext(nc) as tc:
        _body(tc, x_d, gwT_d, vwT_d, owT_d, sel_d, out_d)
    nc.compile()
    return nc


def _body(tc, x_d, gwT_d, vwT_d, owT_d, sel_d, out_d):
    nc = tc.nc
    gp = nc.gpsimd

    with (
        tc.tile_pool(name="pp", bufs=1) as pp,
        tc.tile_pool(name="psp", bufs=8, space="PSUM") as psp,
        tc.tile_pool(name="drp", bufs=1, space="DRAM") as drp,
    ):
        # ---------- DRAM scratch ----------
        grb_i = drp.tile([1, 4], F32, tag="grb_i")
        grb_o = drp.tile([1, 4], F32, tag="grb_o", addr_space="Shared")
        w2b = drp.tile([HL, D], BF16, tag="w2b")                # AG in
        w2g = drp.tile([N_CORES * HL, D], BF16, tag="w2g",
                       addr_space="Shared")                      # AG out
        hsp = [drp.tile([KHL, 128, HL], F32, tag=f"hsp{r}", name=f"hsp{r}")
               for r in range(N_CORES)]
        arh_i = [drp.tile([1, TL], F32, tag=f"arhi{r}", name=f"arhi{r}")
                 for r in range(N_CORES)]
        arh_o = [drp.tile([1, TL], F32, tag=f"arho{r}", name=f"arho{r}",
                          addr_space="Shared") for r in range(N_CORES)]
        a2i = [drp.tile([N_CORES, 128, HL], BF16, tag=f"a2i{r}",
                        name=f"a2i{r}") for r in range(N_CORES)]
        a2o = [drp.tile([N_CORES, 128, HL], BF16, tag=f"a2o{r}",
                        name=f"a2o{r}") for r in range(N_CORES)]

        # ---------- persistent SBUF (whole program) ----------
        gam = pp.tile([128, 4], F32, tag="gam")             # g, v, o gammas
        thr = pp.tile([128, 6], F32, tag="thr")             # +-thr g/v/o
        gxall = pp.tile([128, 64], F32, tag="gxall")        # gamma_x per tok
        s1a = pp.tile([128, 64], F32, tag="s1a")
        s2a = pp.tile([128, 64], F32, tag="s2a")
        s12a = pp.tile([128, 64], F32, tag="s12a")
        selb = pp.tile([128, N_CORES], F32, tag="selb")
        sofull = [pp.tile([128, KHL], F32, tag=f"sofull{r}",
                          name=f"sofull{r}") for r in range(N_CORES)]
        sosel = [pp.tile([128, 1], F32, tag=f"sosel{r}", name=f"sosel{r}")
                 for r in range(N_CORES)]
        # mm2 lhsT, k-half 1 (hidden sources j=0..3): assembled during mm1
        khT1 = pp.tile([128, KH // 2, TL], BF16, tag="khT1")    # 8.4 MB

        Gv = gwT_d.ap().rearrange("(c p) h -> c p h", p=128)    # 16 x [128,HL]
        Vv = vwT_d.ap().rearrange("(c p) h -> c p h", p=128)
        Ov = owT_d.ap().rearrange("(c p) d -> c p d", p=128)    # 8 x [128,D]
        Xv = x_d.ap().rearrange("(m p) d -> m p d", p=128)      # 64 x [128,D]

        # ============ pass A: |w| sums -> tiny AllReduce -> gammas =======
        with tc.tile_pool(name="wpa", bufs=3) as wpa:
            parts = wpa.tile([128, 4 * KD], F32, tag="parts", bufs=1)
            srcs = ([(Gv[c], c) for c in range(KD)]
                    + [(Vv[c], KD + c) for c in range(KD)]
                    + [(Ov[c][:, hf * HL:(hf + 1) * HL],
                        2 * KD + 2 * c + hf)
                       for c in range(KHL) for hf in range(2)])
            for src, col in srcs:
                wt = wpa.tile([128, HL], F32, tag="ga_in", bufs=6)
                nc.sync.dma_start(out=wt[:, :], in_=src)
                scr = wpa.tile([128, HL], F32, tag="ga_scr", bufs=2)
                nc.scalar.activation(out=scr[:, :], in_=wt[:, :],
                                     func=AF.Abs,
                                     accum_out=parts[:, col:col + 1])
            gsum = wpa.tile([128, 4], F32, tag="gsum", bufs=1)
            nc.vector.memset(gsum[:, :], 0.0)
            for j, sl in enumerate((slice(0, KD), slice(KD, 2 * KD),
                                    slice(2 * KD, 2 * KD + 2 * KHL))):
                red = wpa.tile([128, 1], F32, tag="red")
                nc.vector.tensor_reduce(out=red[:, :], in_=parts[:, sl],
                                        axis=AX.X, op=OP.add)
                gp.partition_all_reduce(gsum[:, j:j + 1], red[:, :], 128,
                                        bass_isa.ReduceOp.add)
            nc.sync.dma_start(out=grb_i[0:1, :], in_=gsum[0:1, :])
            gp.collective_compute("AllReduce", OP.add, replica_groups=RG,
                                  ins=[grb_i[:, :].opt()],
                                  outs=[grb_o[:, :].opt()])
            g0 = wpa.tile([1, 4], F32, tag="g0")
            nc.sync.dma_start(out=g0[:, :], in_=grb_o[0:1, :])
            gbc = wpa.tile([128, 4], F32, tag="gbc")
            gp.partition_broadcast(gbc[:, :], g0[:, :])
            # gamma = clip(sum / (H*D), 1e-5); same count for all 3
            nc.vector.tensor_scalar(out=gam[:, :], in0=gbc[:, :],
                                    scalar1=1.0 / (H * D), scalar2=1e-5,
                                    op0=OP.mult, op1=OP.max)
            for j in range(3):
                nc.vector.tensor_scalar_mul(
                    out=thr[:, 2 * j:2 * j + 1], in0=gam[:, j:j + 1],
                    scalar1=0.5)
                nc.vector.tensor_scalar_mul(
                    out=thr[:, 2 * j + 1:2 * j + 2], in0=gam[:, j:j + 1],
                    scalar1=-0.5)
            # sel8 one-hot -> all partitions
            s0 = wpa.tile([1, N_CORES], F32, tag="s0")
            nc.sync.dma_start(out=s0[:, :], in_=sel_d.ap())
            gp.partition_broadcast(selb[:, :], s0[:, :])

        thr_g, nthr_g = thr[:, 0:1], thr[:, 1:2]
        thr_v, nthr_v = thr[:, 2:3], thr[:, 3:4]
        thr_o, nthr_o = thr[:, 4:5], thr[:, 5:6]

        def tern_act(pool, wt, out_ap, thr_p, thr_n):
            sp = pool.tile([128, HL], BF16, tag="q_sp")
            nc.scalar.activation(out=sp[:, :], in_=wt[:, :],
                                 func=AF.Sign, bias=thr_n)
            sn = pool.tile([128, HL], BF16, tag="q_sn")
            nc.scalar.activation(out=sn[:, :], in_=wt[:, :],
                                 func=AF.Sign, bias=thr_p)
            nc.vector.tensor_add(out=out_ap, in0=sp[:, :], in1=sn[:, :])

        def tern_dve(pool, wt, out_ap, thr_p, thr_n):
            mp = pool.tile([128, HL], BF16, tag="q_sp")
            nc.vector.tensor_scalar(out=mp[:, :], in0=wt[:, :],
                                    scalar1=thr_p, scalar2=2.0,
                                    op0=OP.is_gt, op1=OP.mult)
            mn = pool.tile([128, HL], BF16, tag="q_sn")
            nc.vector.tensor_scalar(out=mn[:, :], in0=wt[:, :],
                                    scalar1=thr_n, scalar2=2.0,
                                    op0=OP.is_lt, op1=OP.mult)
            nc.vector.tensor_sub(out=out_ap, in0=mp[:, :], in1=mn[:, :])

        with tc.tile_pool(name="rq", bufs=2) as rq:
            # rq spans M1+M2 (the r=7 requant is emitted mid-mm2; keeping
            # the pool open prevents mm2 pools from reusing its addresses
            # and re-serializing behind the requant)

            def requant_block(r):
                """h block r -> exact global per-token max -> bf16
                integer levels -> AllToAll."""
                ghr = rq.tile([128, KHL], F32, tag="ghr", name=f"ghr{r}")
                nc.sync.dma_start(
                    out=ghr[:, :],
                    in_=arh_o[r][0, :].rearrange("(ml p) -> p ml", p=128))
                gcl = rq.tile([128, KHL], F32, tag="gcl", name=f"gcl{r}")
                nc.vector.tensor_scalar_max(out=gcl[:, :], in0=ghr[:, :],
                                            scalar1=1e-5)
                nc.vector.tensor_scalar(out=sofull[r][:, :], in0=gcl[:, :],
                                        scalar1=gam[:, 2:3],
                                        scalar2=1.0 / 254.0,
                                        op0=OP.mult, op1=OP.mult)
                solm = rq.tile([128, KHL], F32, tag="solm", name=f"solm{r}")
                nc.vector.tensor_mul(out=solm[:, :], in0=sofull[r][:, :],
                                     in1=selb[:, :])
                nc.vector.tensor_reduce(out=sosel[r][:, :], in_=solm[:, :],
                                        axis=AX.X, op=OP.add)
                rcph = rq.tile([128, KHL], F32, tag="rcph", name=f"rcph{r}")
                nc.vector.reciprocal(out=rcph[:, :], in_=gcl[:, :])
                shr = rq.tile([128, KHL], F32, tag="shr", name=f"shr{r}")
                nc.vector.tensor_scalar_mul(out=shr[:, :], in0=rcph[:, :],
                                            scalar1=127.0)
                for ml in range(KHL):
                    for hf in range(2):
                        hld = rq.tile([128, HL // 2], F32, tag="hld")
                        nc.sync.dma_start(
                            out=hld[:, :],
                            in_=hsp[r][ml, :,
                                       hf * (HL // 2):(hf + 1) * (HL // 2)])
                        hmg = rq.tile([128, HL // 2], F32, tag="hmg")
                        nc.scalar.activation(out=hmg[:, :], in_=hld[:, :],
                                             func=AF.Copy,
                                             scale=shr[:, ml:ml + 1],
                                             bias=MAGIC)
                        kh = rq.tile([128, HL // 2], BF16, tag="kh")
                        nc.vector.tensor_scalar_sub(out=kh[:, :],
                                                    in0=hmg[:, :],
                                                    scalar1=MAGIC)
                        nc.sync.dma_start(
                            out=a2i[r][ml, :,
                                       hf * (HL // 2):(hf + 1) * (HL // 2)],
                            in_=kh[:, :])
                gp.collective_compute("AllToAll", OP.bypass,
                                      replica_groups=RG,
                                      ins=[a2i[r][:, :, :].opt()],
                                      outs=[a2o[r][:, :, :].opt()])

            with tc.tile_pool(name="wW", bufs=1) as wW:
                WgT = wW.tile([128, KD, HL], BF16, tag="WgT")   # 4.2 MB
                WvT = wW.tile([128, KD, HL], BF16, tag="WvT")   # 4.2 MB
                # ---- pass B: ternarize gate/val, then out_w ----
                with tc.tile_pool(name="wpb", bufs=3) as wpb:
                    for c in range(KD):
                        wt = wpb.tile([128, HL], F32, tag="q_in", bufs=6)
                        nc.sync.dma_start(out=wt[:, :], in_=Gv[c])
                        tern_act(wpb, wt, WgT[:, c, :], thr_g, nthr_g)
                        wtv = wpb.tile([128, HL], F32, tag="q_in", bufs=6)
                        nc.sync.dma_start(out=wtv[:, :], in_=Vv[c])
                        tern_dve(wpb, wtv, WvT[:, c, :], thr_v, nthr_v)
                    for c in range(KHL):
                        for hf in range(2):
                            wt = wpb.tile([128, HL], F32, tag="q_in",
                                          bufs=6)
                            nc.sync.dma_start(
                                out=wt[:, :],
                                in_=Ov[c][:, hf * HL:(hf + 1) * HL])
                            tq = wpb.tile([128, HL], BF16, tag="q_tq")
                            tern = tern_act if hf == 0 else tern_dve
                            tern(wpb, wt, tq[:, :], thr_o, nthr_o)
                            nc.sync.dma_start(
                                out=w2b[c * 128:(c + 1) * 128,
                                        hf * HL:(hf + 1) * HL],
                                in_=tq[:, :])
                gp.collective_compute("AllGather", OP.bypass,
                                      replica_groups=RG,
                                      ins=[w2b[:, :].opt()],
                                      outs=[w2g[:, :].opt()])

                # ============ phase M1: x-quant + mm1 + requant + A2A ====
                with (
                    tc.tile_pool(name="kxp", bufs=2) as kxp,
                    tc.tile_pool(name="xq", bufs=2) as xq,
                    tc.tile_pool(name="m1e", bufs=2) as m1e,
                ):
                    for r in range(N_CORES):
                        hmall = m1e.tile([128, KHL], F32, tag="hmall",
                                         bufs=2, name=f"hmall{r}")
                        for qb in range(4):         # 256-token quarters
                            kxq = kxp.tile([128, KD, 256], BF16, tag="kxq")
                            for j in range(2):
                                m = r * 8 + qb * 2 + j
                                # quantize token chunk m (two 1024-wide
                                # halves of d) into kxq[:, :, j*128...]
                                gx2 = xq.tile([128, 2], F32, tag="gx2")
                                for hf in range(2):
                                    xt = xq.tile([128, D // 2], F32,
                                                 tag="x_in")
                                    nc.sync.dma_start(
                                        out=xt[:, :],
                                        in_=Xv[m][:, hf * (D // 2):
                                                  (hf + 1) * (D // 2)])
                                    nc.vector.tensor_reduce(
                                        out=gx2[:, hf:hf + 1], in_=xt[:, :],
                                        axis=AX.X, op=OP.max,
                                        apply_absolute_value=True)
                                    if hf == 0:
                                        xt0 = xt
                                gxm = gxall[:, m:m + 1]
                                gmx = xq.tile([128, 1], F32, tag="gmx")
                                nc.vector.tensor_max(out=gmx[:, :],
                                                     in0=gx2[:, 0:1],
                                                     in1=gx2[:, 1:2])
                                nc.vector.tensor_scalar_max(out=gxm,
                                                            in0=gmx[:, :],
                                                            scalar1=1e-5)
                                rcp = xq.tile([128, 1], F32, tag="rcpx")
                                nc.vector.reciprocal(out=rcp[:, :], in_=gxm)
                                sx = xq.tile([128, 1], F32, tag="sx")
                                nc.vector.tensor_scalar_mul(out=sx[:, :],
                                                            in0=rcp[:, :],
                                                            scalar1=127.0)
                                for hf, xth in ((0, xt0), (1, xt)):
                                    xs = xq.tile([128, D // 2], F32,
                                                 tag="x_sc")
                                    nc.vector.tensor_scalar(
                                        out=xs[:, :], in0=xth[:, :],
                                        scalar1=sx[:, :], scalar2=MAGIC,
                                        op0=OP.mult, op1=OP.add)
                                    kx = xq.tile([128, D // 2], BF16,
                                                 tag="kx")
                                    nc.vector.tensor_scalar_sub(
                                        out=kx[:, :], in0=xs[:, :],
                                        scalar1=MAGIC)
                                    nc.sync.dma_start(
                                        out=kxq[:, hf * 8:(hf + 1) * 8,
                                                j * 128:(j + 1) * 128],
                                        in_=kx[:, :], transpose=True)
                                # per-token eviction scales for chunk m
                                nc.vector.tensor_scalar(
                                    out=s1a[:, m:m + 1], in0=gxm,
                                    scalar1=gam[:, 0:1],
                                    scalar2=1.0 / 254.0,
                                    op0=OP.mult, op1=OP.mult)
                                nc.vector.tensor_scalar(
                                    out=s2a[:, m:m + 1], in0=gxm,
                                    scalar1=gam[:, 1:2],
                                    scalar2=1.0 / 254.0,
                                    op0=OP.mult, op1=OP.mult)
                                nc.vector.tensor_mul(
                                    out=s12a[:, m:m + 1],
                                    in0=s1a[:, m:m + 1],
                                    in1=s2a[:, m:m + 1])
                            for j in range(2):
                                ml = qb * 2 + j
                                m = r * 8 + ml
                                hm2 = m1e.tile([128, 2], F32, tag="hm2")
                                # ps = [pg(n=0), pv(n=0), pg(n=1), pv(n=1)]
                                ps = [psp.tile([128, 512], F32, tag="ps",
                                               name=f"ps{m}_{i}")
                                      for i in range(4)]
                                for k in range(KD):
                                    lhsT = kxq[:, k, j * 128:(j + 1) * 128]
                                    for i, (w, n) in enumerate(
                                            ((WgT, 0), (WvT, 0), (WgT, 1),
                                             (WvT, 1))):
                                        nc.tensor.matmul(
                                            ps[i][:, :], lhsT=lhsT,
                                            rhs=w[:, k,
                                                  n * 512:(n + 1) * 512],
                                            start=(k == 0),
                                            stop=(k == KD - 1))
                                for n in range(2):
                                    pg, pv = ps[2 * n], ps[2 * n + 1]
                                    A = m1e.tile([128, 512], F32,
                                                 tag="Asb")
                                    nc.scalar.activation(
                                        out=A[:, :], in_=pg[:, :],
                                        func=AF.Sigmoid,
                                        scale=s1a[:, m:m + 1])
                                    t1 = m1e.tile([128, 512], F32,
                                                  tag="t1sb")
                                    nc.vector.scalar_tensor_tensor(
                                        out=t1[:, :], in0=pg[:, :],
                                        scalar=s12a[:, m:m + 1],
                                        in1=A[:, :],
                                        op0=OP.mult, op1=OP.mult)
                                    hs = m1e.tile([128, 512], F32,
                                                  tag="hssb")
                                    nc.vector.tensor_mul(out=hs[:, :],
                                                         in0=pv[:, :],
                                                         in1=t1[:, :])
                                    nc.vector.tensor_reduce(
                                        out=hm2[:, n:n + 1], in_=hs[:, :],
                                        axis=AX.X, op=OP.max,
                                        apply_absolute_value=True)
                                    nc.sync.dma_start(
                                        out=hsp[r][ml, :,
                                                   n * 512:(n + 1) * 512],
                                        in_=hs[:, :])
                                nc.vector.tensor_max(
                                    out=hmall[:, ml:ml + 1],
                                    in0=hm2[:, 0:1], in1=hm2[:, 1:2])
                        nc.sync.dma_start(
                            out=arh_i[r][0, :].rearrange("(ml p) -> p ml",
                                                         p=128),
                            in_=hmall[:, :])
                        gp.collective_compute("AllReduce", OP.max,
                                              replica_groups=RG,
                                              ins=[arh_i[r][:, :].opt()],
                                              outs=[arh_o[r][:, :].opt()])
                        if r < RLAST:
                            requant_block(r)
                            # assemble mm2 lhsT k-half 1 straight from the
                            # AllToAll output (transpose-DMA from DRAM)
                            for j in range(N_CORES // 2):
                                nc.sync.dma_start(
                                    out=khT1[:, j * KHL:(j + 1) * KHL,
                                             r * 128:(r + 1) * 128],
                                    in_=a2o[r][j], transpose=True)

            # ================= phase M2: mm2 =================
            with (
                tc.tile_pool(name="m2p", bufs=1) as m2p,
                tc.tile_pool(name="m2w", bufs=4) as m2w,
                tc.tile_pool(name="m2o", bufs=4) as m2o,
            ):
                khT2 = m2p.tile([128, KH // 2, TL], BF16,
                                tag="khT2")                 # 8.4 MB
                for r in range(RLAST):
                    for j in range(N_CORES // 2, N_CORES):
                        nc.sync.dma_start(
                            out=khT2[:, (j - 4) * KHL:(j - 3) * KHL,
                                     r * 128:(r + 1) * 128],
                            in_=a2o[r][j], transpose=True)
                Wo = w2g[:, :].rearrange("(k p) d -> k p d", p=128)
                Outv = out_d.ap().rearrange("(r p) d -> r p d", p=128)

                def evict(po_r, r, dcol):
                    ot = m2o.tile([128, 512], F32, tag="ot")
                    nc.scalar.activation(out=ot[:, :], in_=po_r[:, :],
                                         func=AF.Copy,
                                         scale=sosel[r][:, :])
                    nc.sync.dma_start(
                        out=Outv[r][:, dcol * 512:(dcol + 1) * 512],
                        in_=ot[:, :])

                def main_pass(dcol):
                    po = [psp.tile([128, 512], F32, tag="ps",
                                   name=f"po{dcol}_{r}")
                          for r in range(RLAST)]
                    for half, kht in ((0, khT1), (1, khT2)):
                        for kl in range(KH // 2):
                            k = half * (KH // 2) + kl
                            w2t = m2w.tile([128, 512], BF16, tag="w2t")
                            nc.sync.dma_start(
                                out=w2t[:, :],
                                in_=Wo[k][:, dcol * 512:(dcol + 1) * 512])
                            for r in range(RLAST):
                                nc.tensor.matmul(
                                    po[r][:, :],
                                    lhsT=kht[:, kl, r * 128:(r + 1) * 128],
                                    rhs=w2t[:, :],
                                    start=(k == 0), stop=(k == KH - 1))
                    for r in range(RLAST):
                        evict(po[r], r, dcol)

                main_pass(0)
                # r=7 requant + its AllToAll: emitted here so its DMAs only
                # queue behind dcol0's weight stream
                requant_block(RLAST)
                for dcol in range(1, 4):
                    main_pass(dcol)
                # tail pass for r=7
                for j in range(N_CORES // 2):
                    nc.sync.dma_start(
                        out=khT1[:, j * KHL:(j + 1) * KHL,
                                 RLAST * 128:(RLAST + 1) * 128],
                        in_=a2o[RLAST][j], transpose=True)
                for j in range(N_CORES // 2, N_CORES):
                    nc.sync.dma_start(
                        out=khT2[:, (j - 4) * KHL:(j - 3) * KHL,
                                 RLAST * 128:(RLAST + 1) * 128],
                        in_=a2o[RLAST][j], transpose=True)
                po7 = [psp.tile([128, 512], F32, tag="ps",
                                name=f"po7_{dc}") for dc in range(4)]
                for half, kht in ((0, khT1), (1, khT2)):
                    for kl in range(KH // 2):
                        k = half * (KH // 2) + kl
                        w2t = m2w.tile([128, D], BF16, tag="w2t7")
                        nc.sync.dma_start(out=w2t[:, :], in_=Wo[k])
                        lhsT = kht[:, kl, RLAST * 128:(RLAST + 1) * 128]
                        for dc in range(4):
                            nc.tensor.matmul(
                                po7[dc][:, :], lhsT=lhsT,
                                rhs=w2t[:, dc * 512:(dc + 1) * 512],
                                start=(k == 0), stop=(k == KH - 1))
                for dc in range(4):
                    evict(po7[dc], RLAST, dc)


_NC_CACHE = {}


def _get_nc():
    if "nc" not in _NC_CACHE:
        _NC_CACHE["nc"] = _build()
    return _NC_CACHE["nc"]


def kernel(x, gate_w, gate_b, val_w, val_b, out_w, out_b, _trace=False):
    x = np.ascontiguousarray(np.asarray(x), dtype=np.float32)
    gate_w = np.asarray(gate_w, dtype=np.float32)
    val_w = np.asarray(val_w, dtype=np.float32)
    out_w = np.asarray(out_w, dtype=np.float32)
    gate_b = np.asarray(gate_b)
    val_b = np.asarray(val_b)
    out_b = np.asarray(out_b)
    assert not np.any(gate_b) and not np.any(val_b), (
        "device kernel folds silu(y+b) with b=0; nonzero gate/val bias "
        "not supported")

    orig_shape = x.shape
    xf = x.reshape(-1, x.shape[-1])
    assert xf.shape == (T, D) and gate_w.shape == (H, D)
    assert val_w.shape == (H, D) and out_w.shape == (D, H)

    nc = _get_nc()
    in_maps = []
    for i in range(N_CORES):
        sel = np.zeros((1, N_CORES), np.float32)
        sel[0, i] = 1.0
        in_maps.append({
            "x": xf,
            "gwT": np.ascontiguousarray(gate_w[i * HL:(i + 1) * HL, :].T),
            "vwT": np.ascontiguousarray(val_w[i * HL:(i + 1) * HL, :].T),
            "owT": np.ascontiguousarray(out_w[:, i * HL:(i + 1) * HL].T),
            "sel8": sel,
        })
    res = run_bass_kernel_spmd(nc, in_maps, core_ids=list(range(N_CORES)),
                               trace=_trace)
    # core i owns tokens r*1024 + i*128 + [0,128) for r in 0..7
    out = np.empty((T, D), np.float32)
    ov = out.reshape(N_CORES, N_CORES, 128, D)       # [r, i, p, d]
    for i in range(N_CORES):
        ov[:, i] = res.results[i]["out"].reshape(N_CORES, 128, D)
    out = out + out_b[None, :].astype(np.float32)
    kernel._last_results = res
    return out.reshape(orig_shape)

